# revision 43
# baseline (speedup 1.0000x reference)
"""Trainium2 Bass kernel for an attention-LSTM decoder (scan over 128 steps).

Data-parallel over batch: 64 batches -> 8 cores x 8 batches. All weights and
the per-core encoder slice live SBUF-resident in bf16; the 128-step recurrence
runs in a For_i loop with feature-major (transposed) activation layouts so
every matmul has its contraction dim on partitions.

Key structure (vs a naive port):
- The output projection W_out is fused into the attention weights
  (scores = Wah0 @ h0 + (W_attn[:, :2H] @ W_out) @ h1 + bias), which takes
  `out` off the recurrence critical path entirely. A flag tile (0 on step
  0, then 1) gates the W_attn @ b_out bias term so step 0 matches the
  reference's zero-initialized `out`.
- All biases are folded into the PSUM accumulations as K=1 matmuls, so the
  activations read gate PSUMs directly.
- All four LSTM gates use a single tanh activation instruction: gates are
  reordered (i,f,o,g) and the g-gate weights are pre-scaled by 2 so
  tanh(0.5*x) yields tanh(g) for the cell gate and the sigmoid building
  block for i/f/o.
- Softmax normalization is deferred to the context vector: ctx_u = expm @
  enc runs unnormalized while the row-sum + reciprocal run in parallel;
  one broadcasted multiply normalizes and casts to bf16.
- The per-step output store is shifted one step (slot s holds
  out(h1[s-1])) so the out-projection overlaps the next step's softmax;
  an epilogue stores the final step.
- A dummy activation before the loop pins the activation-function table so
  no LoadActFuncSet lands inside the loop body.

Self-contained: hardcodes all shapes; imports the Bass/Tile stack from the
machine-wide /opt/trn_rl_repo checkout.
"""
import sys

sys.path.insert(0, "/opt/trn_rl_repo")
import contextlib

import ml_dtypes
import numpy as np

import concourse.bacc as bacc
import concourse.bass as bass
import concourse.tile as tile
from concourse import mybir

import hashlib


def _bust_dim(*args):
    """PJRT's NEFF cache fingerprints the HLO without the custom call's
    backend_config (where the Bass BIR lives), so different kernels with the
    same I/O signature alias to one cached NEFF. Encode a hash of this file +
    the build args into an (otherwise unused) input tensor's shape so every
    kernel revision gets a distinct cache key."""
    h = hashlib.sha256()
    try:
        with open(__file__, "rb") as f:
            h.update(f.read())
    except OSError:
        pass
    h.update(repr(args).encode())
    return int.from_bytes(h.digest()[:4], "little") % 251 + 1


B, ENC, DEC, H = 64, 256, 128, 512
IN = 2 * H
NCORES = 8
BL = B // NCORES  # 8 batches per core

F32 = mybir.dt.float32
BF16 = mybir.dt.bfloat16
TANH = mybir.ActivationFunctionType.Tanh
EXP = mybir.ActivationFunctionType.Exp
MULT = mybir.AluOpType.mult
ADD = mybir.AluOpType.add


def build_nc(dec=DEC, unroll=8, loop_mult=1):
    nc = bacc.Bacc("TRN2", num_devices=NCORES, debug=False)

    d_wi0 = nc.dram_tensor("wi0", [128, 8 * 4 * H], BF16, kind="ExternalInput")
    d_wh0 = nc.dram_tensor("wh0", [128, 4 * 4 * H], BF16, kind="ExternalInput")
    d_wi1 = nc.dram_tensor("wi1", [128, 4 * 4 * H], BF16, kind="ExternalInput")
    d_wh1 = nc.dram_tensor("wh1", [128, 4 * 4 * H], BF16, kind="ExternalInput")
    d_wat = nc.dram_tensor("wat", [128, 8 * ENC], BF16, kind="ExternalInput")
    d_wou = nc.dram_tensor("wou", [128, 4 * IN], BF16, kind="ExternalInput")
    d_enc = nc.dram_tensor("enc", [128, BL * 2 * IN], BF16, kind="ExternalInput")
    d_msk = nc.dram_tensor("msk", [128, 16], F32, kind="ExternalInput")
    d_b0 = nc.dram_tensor("b0", [16, 128], BF16, kind="ExternalInput")
    d_b1 = nc.dram_tensor("b1", [16, 128], BF16, kind="ExternalInput")
    d_batt = nc.dram_tensor("batt", [2, 128], BF16, kind="ExternalInput")
    d_wab = nc.dram_tensor("wab", [1, ENC], BF16, kind="ExternalInput")
    d_i16 = nc.dram_tensor("i16", [16, 128], BF16, kind="ExternalInput")
    d_i2 = nc.dram_tensor("i2", [2, 16], BF16, kind="ExternalInput")
    d_bo = nc.dram_tensor("bo", [128, 64], F32, kind="ExternalInput")
    nc.dram_tensor(
        "bust", [1, _bust_dim(dec, unroll, loop_mult)], F32, kind="ExternalInput"
    )
    # y[g, tc, p, t'*8+b] = out(h1[32*tc+t'])[b, g*128+p]
    TCH = max(dec // 32, 1)  # t-chunks of 32 steps (256-col matmul sweeps)
    TW = dec // TCH
    d_y = nc.dram_tensor("y", [8, TCH, 128, TW * 8], F32, kind="ExternalOutput")

    with tile.TileContext(nc) as tc:
        with contextlib.ExitStack() as ctx:
            cpool = ctx.enter_context(tc.tile_pool(name="cpool", bufs=1))
            state = ctx.enter_context(tc.tile_pool(name="state", bufs=1))
            work = ctx.enter_context(tc.tile_pool(name="work", bufs=2))
            psum = ctx.enter_context(tc.tile_pool(name="psum", bufs=1, space="PSUM"))

            # ---- load constants ----
            def load(dram, shape, dtype, nsplit=1, tag=None):
                t = cpool.tile(shape, dtype, tag=tag)
                cols = shape[1]
                step = cols // nsplit
                for i in range(nsplit):
                    nc.gpsimd.dma_start(
                        t[:, i * step : (i + 1) * step],
                        dram[:, i * step : (i + 1) * step],
                    )
                return t

            wi0 = load(d_wi0, [128, 8 * 4 * H], BF16, nsplit=4, tag="wi0")
            wh0 = load(d_wh0, [128, 4 * 4 * H], BF16, nsplit=2, tag="wh0")
            wi1 = load(d_wi1, [128, 4 * 4 * H], BF16, nsplit=2, tag="wi1")
            wh1 = load(d_wh1, [128, 4 * 4 * H], BF16, nsplit=2, tag="wh1")
            wat = load(d_wat, [128, 8 * ENC], BF16, tag="wat")
            wou = load(d_wou, [128, 4 * IN], BF16, tag="wou")
            enc = load(d_enc, [128, BL * 2 * IN], BF16, nsplit=4, tag="enc")
            msk = load(d_msk, [128, 16], F32, tag="msk")
            b0 = load(d_b0, [16, 128], BF16, tag="b0")
            b1 = load(d_b1, [16, 128], BF16, tag="b1")
            batt = load(d_batt, [2, 128], BF16, tag="batt")
            wab = load(d_wab, [1, ENC], BF16, tag="wab")
            i16 = load(d_i16, [16, 128], BF16, tag="i16")
            i2 = load(d_i2, [2, 16], BF16, tag="i2")
            bo = load(d_bo, [128, 64], F32, tag="bo")
            ones128 = cpool.tile([128, 128], BF16)
            nc.vector.memset(ones128, 1.0)

            # ---- recurrent state (feature-major) ----
            c0 = state.tile([128, 32], F32)
            c1 = state.tile([128, 32], F32)
            h0 = state.tile([128, 32], BF16)
            h1 = state.tile([128, 32], BF16)
            flag = state.tile([1, 8], BF16)
            for t in (c0, c1, h0, h1):
                nc.vector.memset(t, 0.0)
            # h1 history for the deferred (post-loop) output projection
            h1h = state.tile([128, dec * 32], BF16)
            # init flag on ACT: doubles as the pre-loop activation that pins
            # the act table (MemsetZero/Exp/Tanh share set 0), so no
            # LoadActFuncSet lands in the loop body. It feeds step 0's score
            # matmuls, so it cannot complete out of order.
            nc.scalar.activation(flag, i2[0:1, 0:8], TANH, scale=0.0)
            # loop PSUM tiles: one instance per tag so the dependency
            # tracker sees a single linear history
            ps_at = psum.tile([128, 32], F32, tag="ps_at")
            ps_g0a = psum.tile([128, 96], F32, tag="ps_g0a")
            ps_g0o = psum.tile([128, 32], F32, tag="ps_g0o")
            ps_g1a = psum.tile([128, 96], F32, tag="ps_g1a")
            ps_g1o = psum.tile([128, 32], F32, tag="ps_g1o")
            ps_ctx = psum.tile([128, 64], F32, tag="ps_ctx")

            def cell_a(ps_a, cT):
                # i,f,g gates (tiles 0..11; g-weights pre-doubled): c update
                th = work.tile([128, 96], F32, tag="th")
                nc.scalar.activation(th, ps_a, TANH, scale=0.5)
                s = work.tile([128, 64], F32, tag="s")
                nc.vector.tensor_scalar(s, th[:, 0:64], 0.5, 0.5, MULT, ADD)
                v = work.tile([128, 32], F32, tag="v")
                nc.vector.tensor_mul(v, s[:, 0:32], th[:, 64:96])
                u = work.tile([128, 32], F32, tag="u")
                nc.vector.tensor_mul(u, s[:, 32:64], cT)
                nc.vector.tensor_add(cT, u, v)
                tc2 = work.tile([128, 32], F32, tag="tc2")
                nc.scalar.activation(tc2, cT, TANH)
                return tc2

            def cell_b(ps_o, tc2, hT):
                # o gate (tiles 12..15): h = sigmoid(o) * tanh(c)
                so = work.tile([128, 32], F32, tag="so")
                nc.scalar.activation(so, ps_o, TANH, scale=0.5)
                so2 = work.tile([128, 32], F32, tag="so2")
                nc.vector.tensor_scalar(so2, so, 0.5, 0.5, MULT, ADD)
                nc.vector.tensor_mul(hT, so2, tc2)

            def gate_bias(ps_a, ps_o, bias):
                # bias[M*128+p] lands at col M*8+b via the block indicator i16
                nc.tensor.matmul(
                    ps_a, lhsT=bias[:, :], rhs=i16[:, 0:96], start=True, stop=False
                )
                nc.tensor.matmul(
                    ps_o, lhsT=bias[:, :], rhs=i16[:, 96:128], start=True,
                    stop=False,
                )

            def gates_w(ps_a, ps_o, w, nk, rhs_of, last, Ms=range(16),
                        stopM=15, j0=0):
                for j in range(j0, nk):
                    rhs = rhs_of(j)
                    for M in Ms:
                        tgt = (
                            ps_a[:, M * 8 : M * 8 + 8]
                            if M < 12
                            else ps_o[:, (M - 12) * 8 : (M - 12) * 8 + 8]
                        )
                        nc.tensor.matmul(
                            tgt,
                            lhsT=w[:, j * 4 * H + 128 * M : j * 4 * H + 128 * M + 128],
                            rhs=rhs,
                            start=False,
                            stop=(last and j == nk - 1 and M == stopM),
                        )

            def scores_early(with_h0):
                # open next step's psc group: biases + Wah0 @ h0
                psc = ps_at[:, 0:16]
                nc.tensor.matmul(psc, lhsT=batt[:, :], rhs=i2[:, :],
                                 start=True, stop=False)
                for mt in range(2):
                    nc.tensor.matmul(
                        psc[:, mt * 8 : mt * 8 + 8],
                        lhsT=wab[0:1, mt * 128 : mt * 128 + 128],
                        rhs=flag[0:1, 0:8],
                        start=False,
                        stop=False,
                    )
                if with_h0:
                    for kc in range(4):
                        rhs = h0[:, kc * 8 : kc * 8 + 8]
                        for mt in range(2):
                            nc.tensor.matmul(
                                psc[:, mt * 8 : mt * 8 + 8],
                                lhsT=wat[:, kc * ENC + 128 * mt : kc * ENC + 128 * mt + 128],
                                rhs=rhs,
                                start=False,
                                stop=False,
                            )

            def step(t_sv):
                # ---- finish attention scores: Wah1 @ h1 (h0 part + biases
                # came from the previous step's tail / prologue) ----
                psc = ps_at[:, 0:16]
                for kc in range(4, 8):
                    rhs = h1[:, (kc % 4) * 8 : (kc % 4) * 8 + 8]
                    for mt in range(2):
                        nc.tensor.matmul(
                            psc[:, mt * 8 : mt * 8 + 8],
                            lhsT=wat[:, kc * ENC + 128 * mt : kc * ENC + 128 * mt + 128],
                            rhs=rhs,
                            start=False,
                            stop=(kc == 7 and mt == 1),
                        )

                # ---- LSTM1 gate bias + Wh1 (cover softmax latency);
                # ps_g0* already hold bias0 + Wh0@h0 from the previous
                # step's tail ----
                gate_bias(ps_g1a, ps_g1o, b1)
                wh1r = lambda j: h1[:, j * 8 : j * 8 + 8]
                gates_w(ps_g1a, ps_g1o, wh1, 4, wh1r, False)

                # ---- masked exp (ACT reads PSUM directly; bias is folded) ----
                expf = work.tile([128, 16], F32, tag="expf")
                nc.scalar.activation(expf, psc, EXP)
                expm = work.tile([128, 16], BF16, tag="expm")
                nc.vector.tensor_mul(expm, expf, msk)
                nc.vector.memset(flag, 1.0)

                # ---- denominators (all partitions) + unnormalized ctx ----
                ps_s = ps_at[:, 16:24]
                nc.tensor.matmul(ps_s, lhsT=ones128, rhs=expm[:, 0:8],
                                 start=True, stop=False)
                nc.tensor.matmul(ps_s, lhsT=ones128, rhs=expm[:, 8:16],
                                 start=False, stop=True)
                # two independent accumulation groups (feature halves) so
                # ctxb/Wi0 on half A overlap the half-B context matmuls
                ctxb = work.tile([128, 64], BF16, tag="ctxb")
                rec = work.tile([128, 8], F32, tag="rec")
                nc.vector.reciprocal(rec, ps_s)
                for half in range(2):
                    dts = range(4 * half, 4 * half + 4)
                    for dt_ in dts:
                        for b in range(BL):
                            for kc in range(2):
                                o = (b * 2 + kc) * IN + 128 * dt_
                                nc.tensor.matmul(
                                    ps_ctx[:, dt_ * 8 + b : dt_ * 8 + b + 1],
                                    lhsT=enc[:, o : o + 128],
                                    rhs=expm[:, kc * 8 + b : kc * 8 + b + 1],
                                    start=(dt_ == dts[0] and b == 0 and kc == 0),
                                    stop=(dt_ == dts[-1] and b == BL - 1
                                          and kc == 1),
                                )
                    ho = 32 * half
                    nc.vector.tensor_mul(
                        ctxb[:, ho : ho + 32].rearrange(
                            "p (dt b) -> p dt b", b=8
                        ),
                        ps_ctx[:, ho : ho + 32].rearrange(
                            "p (dt b) -> p dt b", b=8
                        ),
                        rec[:, :].unsqueeze(1).broadcast_to([128, 4, 8]),
                    )

                # ---- LSTM0: Wi0 @ ctx (i,f,g first), split cell ----
                rhs0 = lambda j: ctxb[:, j * 8 : j * 8 + 8]
                gates_w(ps_g0a, ps_g0o, wi0, 4, rhs0, False, Ms=range(12))
                gates_w(
                    ps_g0a, ps_g0o, wi0, 8, rhs0, True, Ms=range(12),
                    stopM=11, j0=4,
                )
                gates_w(ps_g0a, ps_g0o, wi0, 8, rhs0, True, Ms=range(12, 16),
                        stopM=15)
                tc20 = cell_a(ps_g0a, c0)
                cell_b(ps_g0o, tc20, h0)

                # ---- LSTM1: Wi1 @ h0 (i,f,g first), split cell ----
                rhs1 = lambda j: h0[:, j * 8 : j * 8 + 8]
                gates_w(ps_g1a, ps_g1o, wi1, 4, rhs1, True, Ms=range(12),
                        stopM=11)
                gates_w(ps_g1a, ps_g1o, wi1, 4, rhs1, True, Ms=range(12, 16),
                        stopM=15)
                # next step's bias0 + Wh0 @ h0(t): streams during cell1
                gate_bias(ps_g0a, ps_g0o, b0)
                wh0r = lambda j: h0[:, j * 8 : j * 8 + 8]
                gates_w(ps_g0a, ps_g0o, wh0, 4, wh0r, False)
                tc21 = cell_a(ps_g1a, c1)
                # open next step's scores (biases + Wah0 @ h0) during cell1
                scores_early(True)
                cell_b(ps_g1o, tc21, h1)
                # stash h1 for the deferred output projection
                nc.vector.tensor_copy(h1h[:, bass.ds(t_sv * 32, 32)], h1)

            scores_early(False)
            gate_bias(ps_g0a, ps_g0o, b0)

            assert dec % unroll == 0
            niter = dec // unroll
            if niter > 1 or loop_mult > 1:
                with tc.For_i(
                    0, niter * loop_mult, hint_engines=(mybir.EngineType.PE,)
                ) as ivr:
                    iv = ivr % niter if loop_mult > 1 else ivr
                    for k in range(unroll):
                        step(iv * unroll + k)
            else:
                for k in range(unroll):
                    step(k)

            # ---- epilogue: batched output projection over all steps ----
            # view h1h as [p, t, kc, b]; rhs per kc = [p, TW t, 8 b] slices
            h1v = h1h[:, :].rearrange("p (t kc b) -> p kc t b", kc=4, b=8)
            outf = cpool.tile([128, 8 * TCH * TW * 8], F32)
            for g in range(8):
                for tc in range(TCH):
                    ps_o = psum.tile([128, TW * 8], F32, tag=f"ps_o{(g * TCH + tc) % 2}")
                    for kc in range(4):
                        nc.tensor.matmul(
                            ps_o,
                            lhsT=wou[:, kc * IN + 128 * g : kc * IN + 128 * g + 128],
                            rhs=h1v[:, kc, tc * TW : (tc + 1) * TW, :],
                            start=(kc == 0),
                            stop=(kc == 3),
                        )
                    o = (g * TCH + tc) * TW * 8
                    nc.vector.tensor_add(
                        outf[:, o : o + TW * 8].rearrange("p (t b) -> p t b", b=8),
                        ps_o[:, :].rearrange("p (t b) -> p t b", b=8),
                        bo[:, g * 8 : g * 8 + 8].unsqueeze(1).broadcast_to(
                            [128, TW, 8]
                        ),
                    )
            nc.gpsimd.dma_start(
                d_y[:, :, :, :].rearrange("g tc p n -> p g tc n"),
                outf[:, :].rearrange(
                    "p (g tc n) -> p g tc n", tc=TCH, n=TW * 8
                ),
            )

    nc.compile()
    return nc


def prep_inputs(inputs):
    """Host-side repack of the reference inputs into the kernel layouts."""
    gi = {k: np.asarray(v, np.float32) for k, v in inputs.items()}
    bf = ml_dtypes.bfloat16

    def reorder(w):
        # torch gate order (i,f,g,o) rows stay (i,f,g,o); g rows doubled
        i, f, g, o = np.split(w, 4, axis=0)
        return np.concatenate([i, f, 2.0 * g, o], axis=0)

    def kmajor(w, nk):
        # w: [M, K] -> [128, nk*M] with [p, kc*M + m] = w[m, 128*kc + p]
        M, K = w.shape
        assert K == nk * 128
        return np.ascontiguousarray(
            w.T.reshape(nk, 128, M).transpose(1, 0, 2).reshape(128, nk * M)
        )

    wah0 = gi["W_attn"][:, 2 * H : 3 * H]                  # [ENC, H]
    wah1 = gi["W_attn"][:, 0 : 2 * H] @ gi["W_out"]        # [ENC, H]
    wat = np.concatenate([kmajor(wah0, 4), kmajor(wah1, 4)], axis=1)
    wab = gi["W_attn"][:, 0 : 2 * H] @ gi["b_out"]         # [ENC]

    shared = {
        "wi0": kmajor(reorder(gi["W_ih0"]), 8).astype(bf),
        "wh0": kmajor(reorder(gi["W_hh0"]), 4).astype(bf),
        "wi1": kmajor(reorder(gi["W_ih1"]), 4).astype(bf),
        "wh1": kmajor(reorder(gi["W_hh1"]), 4).astype(bf),
        "wat": wat.astype(bf),
        "wou": kmajor(gi["W_out"], 4).astype(bf),
        "b0": reorder((gi["b_ih0"] + gi["b_hh0"]).reshape(4 * H, 1))
        .reshape(16, 128)
        .astype(bf),
        "b1": reorder((gi["b_ih1"] + gi["b_hh1"]).reshape(4 * H, 1))
        .reshape(16, 128)
        .astype(bf),
        "batt": gi["b_attn"].reshape(2, 128).astype(bf),
        "wab": wab.reshape(1, ENC).astype(bf),
        "i16": np.repeat(np.eye(16, dtype=np.float32), 8, axis=1).astype(bf),
        "i2": np.repeat(np.eye(2, dtype=np.float32), 8, axis=1).astype(bf),
        "bo": np.repeat(gi["b_out"].reshape(8, 128).T, 8, axis=1).astype(
            np.float32
        ),
    }
    in_maps = []
    for c in range(NCORES):
        e = gi["encoder2_hiddens"][c * BL : (c + 1) * BL]  # [8, 256, 1024]
        enc_t = np.ascontiguousarray(
            e.reshape(BL, 2, 128, IN).transpose(2, 0, 1, 3).reshape(128, BL * 2 * IN)
        ).astype(bf)
        m = gi["x2_mask"][c * BL : (c + 1) * BL]  # [8, 256] int32
        mf = (1 - m).astype(np.float32).T  # [256, 8]
        msk = np.ascontiguousarray(
            mf.reshape(2, 128, BL).transpose(1, 0, 2).reshape(128, 16)
        )
        in_maps.append({**shared, "enc": enc_t, "msk": msk})
    return in_maps


def decode_y(arr, dec=DEC):
    """[8, TCH, 128, TW*8] per-core DRAM layout -> [BL, dec, IN]."""
    tch = max(dec // 32, 1)
    tw = dec // tch
    a = arr.reshape(8, tch, 128, tw, 8)        # (g, tc, p, t', b)
    # y[b, t, g*128+p] with t = tc*tw + t'
    return np.ascontiguousarray(
        a.transpose(4, 1, 3, 0, 2)             # (b, tc, t', g, p)
    ).reshape(BL, dec, IN)


_cache = {}


def _get_nc(dec=DEC, unroll=8, loop_mult=1):
    key = (dec, unroll, loop_mult)
    if key not in _cache:
        _cache[key] = build_nc(dec, unroll, loop_mult)
    return _cache[key]


class Runner:
    """Jit-compiles the Bass program once; repeat calls reuse the executable
    and the device-resident input shards (only fresh output buffers are
    shipped per call when donation is enabled)."""

    def __init__(self, nc, donate=True):
        import jax
        from concourse import bass2jax
        from jax.experimental.shard_map import shard_map
        from jax.sharding import Mesh, PartitionSpec

        bass2jax.install_neuronx_cc_hook()
        self.jax = jax
        self.nc = nc
        self.donate = donate
        pname = nc.partition_id_tensor.name if nc.partition_id_tensor else None
        in_names, out_names, out_avals, zero_outs = [], [], [], []
        self.in_shapes = {}
        for alloc in nc.m.functions[0].allocations:
            if not isinstance(alloc, mybir.MemoryLocationSet):
                continue
            name = alloc.memorylocations[0].name
            if alloc.kind == "ExternalInput":
                if name != pname:
                    in_names.append(name)
                    self.in_shapes[name] = (
                        tuple(alloc.tensor_shape),
                        mybir.dt.np(alloc.dtype),
                    )
            elif alloc.kind == "ExternalOutput":
                shape = tuple(alloc.tensor_shape)
                dtype = mybir.dt.np(alloc.dtype)
                out_names.append(name)
                out_avals.append(jax.core.ShapedArray(shape, dtype))
                zero_outs.append(np.zeros(shape, dtype))
        self.in_names, self.out_names = in_names, out_names
        self.out_avals, self.zero_outs = out_avals, zero_outs
        n_params, n_outs = len(in_names), len(out_names)
        all_names = in_names + out_names + ([pname] if pname else [])

        def _body(*args):
            operands = list(args)
            if pname is not None:
                operands.append(bass2jax.partition_id_tensor())
            outs = bass2jax._bass_exec_p.bind(
                *operands,
                out_avals=tuple(out_avals),
                in_names=tuple(all_names),
                out_names=tuple(out_names),
                lowering_input_output_aliases=(),
                sim_require_finite=True,
                sim_require_nnan=True,
                nc=nc,
            )
            return tuple(outs)

        devices = jax.devices()[:NCORES]
        assert len(devices) == NCORES
        self.mesh = Mesh(np.asarray(devices), ("core",))
        in_specs = (PartitionSpec("core"),) * (n_params + n_outs)
        out_specs = (PartitionSpec("core"),) * n_outs
        kw = (
            dict(donate_argnums=tuple(range(n_params, n_params + n_outs)))
            if donate
            else {}
        )
        self.fn = jax.jit(
            shard_map(
                _body, mesh=self.mesh, in_specs=in_specs, out_specs=out_specs,
                check_rep=False,
            ),
            keep_unused=True,
            **kw,
        )
        self._dev_in = None

    def _globalize(self, in_maps):
        jax = self.jax
        from jax.sharding import NamedSharding, PartitionSpec

        sh = NamedSharding(self.mesh, PartitionSpec("core"))
        arrs = []
        for name in self.in_names:
            if name in in_maps[0]:
                g = np.concatenate(
                    [np.asarray(m[name]) for m in in_maps], axis=0
                )
            else:  # auto-fill (e.g. the cache-bust tensor)
                shape, dt = self.in_shapes[name]
                g = np.zeros((NCORES * shape[0], *shape[1:]), dt)
            arrs.append(jax.device_put(g, sh))
        return arrs

    def set_inputs(self, in_maps):
        self._dev_in = self._globalize(in_maps)

    def _zeros_dev(self):
        from jax.sharding import NamedSharding, PartitionSpec

        sh = NamedSharding(self.mesh, PartitionSpec("core"))
        return [
            self.jax.device_put(
                np.zeros((NCORES * z.shape[0], *z.shape[1:]), z.dtype), sh
            )
            for z in self.zero_outs
        ]

    def __call__(self):
        outs = self.fn(*self._dev_in, *self._zeros_dev())
        return outs

    def gather(self, outs):
        res = []
        for i, name in enumerate(self.out_names):
            a = np.asarray(outs[i])
            res.append(a.reshape(NCORES, *self.out_avals[i].shape))
        return dict(zip(self.out_names, res))


_runner_cache = {}


def get_runner(dec=DEC, unroll=None, donate=True, loop_mult=1):
    unroll = CFG["unroll"] if unroll is None else unroll
    key = (dec, unroll, donate, loop_mult)
    if key not in _runner_cache:
        _runner_cache[key] = Runner(_get_nc(dec, unroll, loop_mult), donate=donate)
    return _runner_cache[key]


CFG = dict(unroll=16)


def run_on_hw(inputs, dec=DEC, unroll=None):
    unroll = CFG["unroll"] if unroll is None else unroll
    r = get_runner(dec, unroll)
    r.set_inputs(prep_inputs(inputs))
    outs = r()
    ys = r.gather(outs)["y"]  # [NCORES, dec+1, 128, 64]
    y = np.concatenate([decode_y(ys[c], dec) for c in range(NCORES)], axis=0)
    return y.astype(np.float32)


def kernel(**inputs):
    return run_on_hw(inputs)


# revision 44
# speedup vs baseline: 170.8242x; 170.8242x over previous
"""Trainium2 Bass kernel for an attention-LSTM decoder (scan over 128 steps).

Data-parallel over batch: 64 batches -> 8 cores x 8 batches. All weights and
the per-core encoder slice live SBUF-resident in bf16; the 128-step recurrence
runs in a For_i loop with feature-major (transposed) activation layouts so
every matmul has its contraction dim on partitions.

Key structure (vs a naive port):
- The output projection W_out is fused into the attention weights
  (scores = Wah0 @ h0 + (W_attn[:, :2H] @ W_out) @ h1 + bias), which takes
  `out` off the recurrence critical path entirely. A flag tile (0 on step
  0, then 1) gates the W_attn @ b_out bias term so step 0 matches the
  reference's zero-initialized `out`.
- All biases are folded into the PSUM accumulations as K=1 matmuls, so the
  activations read gate PSUMs directly.
- All four LSTM gates use a single tanh activation instruction: gates are
  reordered (i,f,o,g) and the g-gate weights are pre-scaled by 2 so
  tanh(0.5*x) yields tanh(g) for the cell gate and the sigmoid building
  block for i/f/o.
- Softmax normalization is deferred to the context vector: ctx_u = expm @
  enc runs unnormalized while the row-sum + reciprocal run in parallel;
  one broadcasted multiply normalizes and casts to bf16.
- The per-step output store is shifted one step (slot s holds
  out(h1[s-1])) so the out-projection overlaps the next step's softmax;
  an epilogue stores the final step.
- A dummy activation before the loop pins the activation-function table so
  no LoadActFuncSet lands inside the loop body.

Self-contained: hardcodes all shapes; imports the Bass/Tile stack from the
machine-wide /opt/trn_rl_repo checkout.
"""
import sys

sys.path.insert(0, "/opt/trn_rl_repo")
import contextlib

import ml_dtypes
import numpy as np

import concourse.bacc as bacc
import concourse.bass as bass
import concourse.tile as tile
from concourse import mybir

import hashlib


def _bust_dim(*args):
    """PJRT's NEFF cache fingerprints the HLO without the custom call's
    backend_config (where the Bass BIR lives), so different kernels with the
    same I/O signature alias to one cached NEFF. Encode a hash of this file +
    the build args into an (otherwise unused) input tensor's shape so every
    kernel revision gets a distinct cache key."""
    h = hashlib.sha256()
    try:
        with open(__file__, "rb") as f:
            h.update(f.read())
    except OSError:
        pass
    h.update(repr(args).encode())
    return int.from_bytes(h.digest()[:4], "little") % 251 + 1


B, ENC, DEC, H = 64, 256, 128, 512
IN = 2 * H
NCORES = 8
BL = B // NCORES  # 8 batches per core

F32 = mybir.dt.float32
BF16 = mybir.dt.bfloat16
TANH = mybir.ActivationFunctionType.Tanh
EXP = mybir.ActivationFunctionType.Exp
MULT = mybir.AluOpType.mult
ADD = mybir.AluOpType.add


def build_nc(dec=DEC, unroll=8, loop_mult=1):
    nc = bacc.Bacc("TRN2", num_devices=NCORES, debug=False)

    d_wi0 = nc.dram_tensor("wi0", [128, 8 * 4 * H], BF16, kind="ExternalInput")
    d_wh0 = nc.dram_tensor("wh0", [128, 4 * 4 * H], BF16, kind="ExternalInput")
    d_wi1 = nc.dram_tensor("wi1", [128, 4 * 4 * H], BF16, kind="ExternalInput")
    d_wh1 = nc.dram_tensor("wh1", [128, 4 * 4 * H], BF16, kind="ExternalInput")
    d_wat = nc.dram_tensor("wat", [128, 8 * ENC], BF16, kind="ExternalInput")
    d_wou = nc.dram_tensor("wou", [128, 4 * IN], BF16, kind="ExternalInput")
    d_enc = nc.dram_tensor("enc", [128, BL * 2 * IN], BF16, kind="ExternalInput")
    d_msk = nc.dram_tensor("msk", [128, 16], F32, kind="ExternalInput")
    d_b0 = nc.dram_tensor("b0", [16, 128], BF16, kind="ExternalInput")
    d_b1 = nc.dram_tensor("b1", [16, 128], BF16, kind="ExternalInput")
    d_batt = nc.dram_tensor("batt", [2, 128], BF16, kind="ExternalInput")
    d_wab = nc.dram_tensor("wab", [1, ENC], BF16, kind="ExternalInput")
    d_i16 = nc.dram_tensor("i16", [16, 128], BF16, kind="ExternalInput")
    d_i2 = nc.dram_tensor("i2", [2, 16], BF16, kind="ExternalInput")
    d_bo = nc.dram_tensor("bo", [128, 64], F32, kind="ExternalInput")
    nc.dram_tensor(
        "bust", [1, _bust_dim(dec, unroll, loop_mult)], F32, kind="ExternalInput"
    )
    # y[g, tc, p, t'*8+b] = out(h1[32*tc+t'])[b, g*128+p]
    TCH = max(dec // 32, 1)  # t-chunks of 32 steps (256-col matmul sweeps)
    TW = dec // TCH
    d_y = nc.dram_tensor("y", [8, TCH, 128, TW * 8], F32, kind="ExternalOutput")

    with tile.TileContext(nc) as tc:
        with contextlib.ExitStack() as ctx:
            cpool = ctx.enter_context(tc.tile_pool(name="cpool", bufs=1))
            state = ctx.enter_context(tc.tile_pool(name="state", bufs=1))
            work = ctx.enter_context(tc.tile_pool(name="work", bufs=2))
            psum = ctx.enter_context(tc.tile_pool(name="psum", bufs=1, space="PSUM"))

            # ---- load constants ----
            def load(dram, shape, dtype, nsplit=1, tag=None):
                t = cpool.tile(shape, dtype, tag=tag)
                cols = shape[1]
                step = cols // nsplit
                for i in range(nsplit):
                    nc.gpsimd.dma_start(
                        t[:, i * step : (i + 1) * step],
                        dram[:, i * step : (i + 1) * step],
                    )
                return t

            wi0 = load(d_wi0, [128, 8 * 4 * H], BF16, nsplit=4, tag="wi0")
            wh0 = load(d_wh0, [128, 4 * 4 * H], BF16, nsplit=2, tag="wh0")
            wi1 = load(d_wi1, [128, 4 * 4 * H], BF16, nsplit=2, tag="wi1")
            wh1 = load(d_wh1, [128, 4 * 4 * H], BF16, nsplit=2, tag="wh1")
            wat = load(d_wat, [128, 8 * ENC], BF16, tag="wat")
            wou = load(d_wou, [128, 4 * IN], BF16, tag="wou")
            enc = load(d_enc, [128, BL * 2 * IN], BF16, nsplit=4, tag="enc")
            msk = load(d_msk, [128, 16], F32, tag="msk")
            b0 = load(d_b0, [16, 128], BF16, tag="b0")
            b1 = load(d_b1, [16, 128], BF16, tag="b1")
            batt = load(d_batt, [2, 128], BF16, tag="batt")
            wab = load(d_wab, [1, ENC], BF16, tag="wab")
            i16 = load(d_i16, [16, 128], BF16, tag="i16")
            i2 = load(d_i2, [2, 16], BF16, tag="i2")
            bo = load(d_bo, [128, 64], F32, tag="bo")
            ones128 = cpool.tile([128, 128], BF16)
            nc.vector.memset(ones128, 1.0)

            # ---- recurrent state (feature-major) ----
            c0 = state.tile([128, 32], F32)
            c1 = state.tile([128, 32], F32)
            h0 = state.tile([128, 32], BF16)
            h1 = state.tile([128, 32], BF16)
            flag = state.tile([1, 8], BF16)
            for t in (c0, c1, h0, h1):
                nc.vector.memset(t, 0.0)
            # h1 history for the deferred (post-loop) output projection
            h1h = state.tile([128, dec * 32], BF16)
            # init flag on ACT: doubles as the pre-loop activation that pins
            # the act table (MemsetZero/Exp/Tanh share set 0), so no
            # LoadActFuncSet lands in the loop body. It feeds step 0's score
            # matmuls, so it cannot complete out of order.
            nc.scalar.activation(flag, i2[0:1, 0:8], TANH, scale=0.0)
            # loop PSUM tiles: one instance per tag so the dependency
            # tracker sees a single linear history
            ps_at = psum.tile([128, 32], F32, tag="ps_at")
            ps_g0a = psum.tile([128, 96], F32, tag="ps_g0a")
            ps_g0o = psum.tile([128, 32], F32, tag="ps_g0o")
            ps_g1a = psum.tile([128, 96], F32, tag="ps_g1a")
            ps_g1o = psum.tile([128, 32], F32, tag="ps_g1o")
            ps_ctx = psum.tile([128, 64], F32, tag="ps_ctx")

            def cell_a(ps_a, cT):
                # i,f,g gates (tiles 0..11; g-weights pre-doubled): c update
                th = work.tile([128, 96], F32, tag="th")
                nc.scalar.activation(th, ps_a, TANH, scale=0.5)
                s = work.tile([128, 64], F32, tag="s")
                nc.vector.tensor_scalar(s, th[:, 0:64], 0.5, 0.5, MULT, ADD)
                v = work.tile([128, 32], F32, tag="v")
                nc.vector.tensor_mul(v, s[:, 0:32], th[:, 64:96])
                u = work.tile([128, 32], F32, tag="u")
                nc.vector.tensor_mul(u, s[:, 32:64], cT)
                nc.vector.tensor_add(cT, u, v)
                tc2 = work.tile([128, 32], F32, tag="tc2")
                nc.scalar.activation(tc2, cT, TANH)
                return tc2

            def cell_b(ps_o, tc2, hT):
                # o gate (tiles 12..15): h = sigmoid(o) * tanh(c)
                so = work.tile([128, 32], F32, tag="so")
                nc.scalar.activation(so, ps_o, TANH, scale=0.5)
                so2 = work.tile([128, 32], F32, tag="so2")
                nc.vector.tensor_scalar(so2, so, 0.5, 0.5, MULT, ADD)
                nc.vector.tensor_mul(hT, so2, tc2)

            def gate_bias(ps_a, ps_o, bias):
                # bias[M*128+p] lands at col M*8+b via the block indicator i16
                nc.tensor.matmul(
                    ps_a, lhsT=bias[:, :], rhs=i16[:, 0:96], start=True, stop=False
                )
                nc.tensor.matmul(
                    ps_o, lhsT=bias[:, :], rhs=i16[:, 96:128], start=True,
                    stop=False,
                )

            def gates_w(ps_a, ps_o, w, nk, rhs_of, last, Ms=range(16),
                        stopM=15, j0=0):
                for j in range(j0, nk):
                    rhs = rhs_of(j)
                    for M in Ms:
                        tgt = (
                            ps_a[:, M * 8 : M * 8 + 8]
                            if M < 12
                            else ps_o[:, (M - 12) * 8 : (M - 12) * 8 + 8]
                        )
                        nc.tensor.matmul(
                            tgt,
                            lhsT=w[:, j * 4 * H + 128 * M : j * 4 * H + 128 * M + 128],
                            rhs=rhs,
                            start=False,
                            stop=(last and j == nk - 1 and M == stopM),
                        )

            def scores_early(with_h0):
                # open next step's psc group: biases + Wah0 @ h0
                psc = ps_at[:, 0:16]
                nc.tensor.matmul(psc, lhsT=batt[:, :], rhs=i2[:, :],
                                 start=True, stop=False)
                for mt in range(2):
                    nc.tensor.matmul(
                        psc[:, mt * 8 : mt * 8 + 8],
                        lhsT=wab[0:1, mt * 128 : mt * 128 + 128],
                        rhs=flag[0:1, 0:8],
                        start=False,
                        stop=False,
                    )
                if with_h0:
                    for kc in range(4):
                        rhs = h0[:, kc * 8 : kc * 8 + 8]
                        for mt in range(2):
                            nc.tensor.matmul(
                                psc[:, mt * 8 : mt * 8 + 8],
                                lhsT=wat[:, kc * ENC + 128 * mt : kc * ENC + 128 * mt + 128],
                                rhs=rhs,
                                start=False,
                                stop=False,
                            )

            def step(t_sv):
                # ---- finish attention scores: Wah1 @ h1 (h0 part + biases
                # came from the previous step's tail / prologue) ----
                psc = ps_at[:, 0:16]
                for kc in range(4, 8):
                    rhs = h1[:, (kc % 4) * 8 : (kc % 4) * 8 + 8]
                    for mt in range(2):
                        nc.tensor.matmul(
                            psc[:, mt * 8 : mt * 8 + 8],
                            lhsT=wat[:, kc * ENC + 128 * mt : kc * ENC + 128 * mt + 128],
                            rhs=rhs,
                            start=False,
                            stop=(kc == 7 and mt == 1),
                        )

                # ---- LSTM1 gate bias + Wh1 (cover softmax latency);
                # ps_g0* already hold bias0 + Wh0@h0 from the previous
                # step's tail ----
                gate_bias(ps_g1a, ps_g1o, b1)
                wh1r = lambda j: h1[:, j * 8 : j * 8 + 8]
                gates_w(ps_g1a, ps_g1o, wh1, 4, wh1r, False)

                # ---- exp (ACT reads PSUM; bias folded). enc is pre-masked
                # on the host so ctx consumes raw expf straight from ACT; the
                # masked expm feeds only the denominator sum. ----
                expf = work.tile([128, 16], BF16, tag="expf")
                nc.scalar.activation(expf, psc, EXP)
                expm = work.tile([128, 16], BF16, tag="expm")
                nc.vector.tensor_mul(expm, expf, msk)
                nc.vector.memset(flag, 1.0)

                # ---- unnormalized ctx in two accumulation groups; the
                # denominator sum (needs masked expm) is emitted after half A
                # so the ctx stream starts straight from expf ----
                ps_s = ps_at[:, 16:24]
                ctxb = work.tile([128, 64], BF16, tag="ctxb")
                rec = work.tile([128, 8], F32, tag="rec")
                for half in range(2):
                    dts = range(4 * half, 4 * half + 4)
                    for dt_ in dts:
                        for b in range(BL):
                            for kc in range(2):
                                o = (b * 2 + kc) * IN + 128 * dt_
                                nc.tensor.matmul(
                                    ps_ctx[:, dt_ * 8 + b : dt_ * 8 + b + 1],
                                    lhsT=enc[:, o : o + 128],
                                    rhs=expf[:, kc * 8 + b : kc * 8 + b + 1],
                                    start=(dt_ == dts[0] and b == 0 and kc == 0),
                                    stop=(dt_ == dts[-1] and b == BL - 1
                                          and kc == 1),
                                )
                    if half == 0:
                        nc.tensor.matmul(ps_s, lhsT=ones128, rhs=expm[:, 0:8],
                                         start=True, stop=False)
                        nc.tensor.matmul(ps_s, lhsT=ones128,
                                         rhs=expm[:, 8:16],
                                         start=False, stop=True)
                        nc.vector.reciprocal(rec, ps_s)
                    ho = 32 * half
                    nc.vector.tensor_mul(
                        ctxb[:, ho : ho + 32].rearrange(
                            "p (dt b) -> p dt b", b=8
                        ),
                        ps_ctx[:, ho : ho + 32].rearrange(
                            "p (dt b) -> p dt b", b=8
                        ),
                        rec[:, :].unsqueeze(1).broadcast_to([128, 4, 8]),
                    )

                # ---- LSTM0: Wi0 @ ctx (i,f,g first), split cell ----
                rhs0 = lambda j: ctxb[:, j * 8 : j * 8 + 8]
                gates_w(ps_g0a, ps_g0o, wi0, 4, rhs0, False, Ms=range(12))
                gates_w(
                    ps_g0a, ps_g0o, wi0, 8, rhs0, True, Ms=range(12),
                    stopM=11, j0=4,
                )
                gates_w(ps_g0a, ps_g0o, wi0, 8, rhs0, True, Ms=range(12, 16),
                        stopM=15)
                tc20 = cell_a(ps_g0a, c0)
                cell_b(ps_g0o, tc20, h0)

                # ---- LSTM1: Wi1 @ h0 (i,f,g first), split cell ----
                rhs1 = lambda j: h0[:, j * 8 : j * 8 + 8]
                gates_w(ps_g1a, ps_g1o, wi1, 4, rhs1, True, Ms=range(12),
                        stopM=11)
                gates_w(ps_g1a, ps_g1o, wi1, 4, rhs1, True, Ms=range(12, 16),
                        stopM=15)
                # next step's bias0 + Wh0 @ h0(t): streams during cell1
                gate_bias(ps_g0a, ps_g0o, b0)
                wh0r = lambda j: h0[:, j * 8 : j * 8 + 8]
                gates_w(ps_g0a, ps_g0o, wh0, 4, wh0r, False)
                tc21 = cell_a(ps_g1a, c1)
                # open next step's scores (biases + Wah0 @ h0) during cell1
                scores_early(True)
                cell_b(ps_g1o, tc21, h1)
                # stash h1 for the deferred output projection
                nc.vector.tensor_copy(h1h[:, bass.ds(t_sv * 32, 32)], h1)

            scores_early(False)
            gate_bias(ps_g0a, ps_g0o, b0)

            assert dec % unroll == 0
            niter = dec // unroll
            if niter > 1 or loop_mult > 1:
                with tc.For_i(
                    0, niter * loop_mult, hint_engines=(mybir.EngineType.PE,)
                ) as ivr:
                    iv = ivr % niter if loop_mult > 1 else ivr
                    for k in range(unroll):
                        step(iv * unroll + k)
            else:
                for k in range(unroll):
                    step(k)

            # ---- epilogue: batched output projection over all steps ----
            # view h1h as [p, t, kc, b]; rhs per kc = [p, TW t, 8 b] slices
            h1v = h1h[:, :].rearrange("p (t kc b) -> p kc t b", kc=4, b=8)
            outf = cpool.tile([128, 8 * TCH * TW * 8], F32)
            for g in range(8):
                for tc in range(TCH):
                    ps_o = psum.tile([128, TW * 8], F32, tag=f"ps_o{(g * TCH + tc) % 2}")
                    for kc in range(4):
                        nc.tensor.matmul(
                            ps_o,
                            lhsT=wou[:, kc * IN + 128 * g : kc * IN + 128 * g + 128],
                            rhs=h1v[:, kc, tc * TW : (tc + 1) * TW, :],
                            start=(kc == 0),
                            stop=(kc == 3),
                        )
                    o = (g * TCH + tc) * TW * 8
                    nc.vector.tensor_add(
                        outf[:, o : o + TW * 8].rearrange("p (t b) -> p t b", b=8),
                        ps_o[:, :].rearrange("p (t b) -> p t b", b=8),
                        bo[:, g * 8 : g * 8 + 8].unsqueeze(1).broadcast_to(
                            [128, TW, 8]
                        ),
                    )
            nc.gpsimd.dma_start(
                d_y[:, :, :, :].rearrange("g tc p n -> p g tc n"),
                outf[:, :].rearrange(
                    "p (g tc n) -> p g tc n", tc=TCH, n=TW * 8
                ),
            )

    nc.compile()
    return nc


def prep_inputs(inputs):
    """Host-side repack of the reference inputs into the kernel layouts."""
    gi = {k: np.asarray(v, np.float32) for k, v in inputs.items()}
    bf = ml_dtypes.bfloat16

    def reorder(w):
        # torch gate order (i,f,g,o) rows stay (i,f,g,o); g rows doubled
        i, f, g, o = np.split(w, 4, axis=0)
        return np.concatenate([i, f, 2.0 * g, o], axis=0)

    def kmajor(w, nk):
        # w: [M, K] -> [128, nk*M] with [p, kc*M + m] = w[m, 128*kc + p]
        M, K = w.shape
        assert K == nk * 128
        return np.ascontiguousarray(
            w.T.reshape(nk, 128, M).transpose(1, 0, 2).reshape(128, nk * M)
        )

    wah0 = gi["W_attn"][:, 2 * H : 3 * H]                  # [ENC, H]
    wah1 = gi["W_attn"][:, 0 : 2 * H] @ gi["W_out"]        # [ENC, H]
    wat = np.concatenate([kmajor(wah0, 4), kmajor(wah1, 4)], axis=1)
    wab = gi["W_attn"][:, 0 : 2 * H] @ gi["b_out"]         # [ENC]

    shared = {
        "wi0": kmajor(reorder(gi["W_ih0"]), 8).astype(bf),
        "wh0": kmajor(reorder(gi["W_hh0"]), 4).astype(bf),
        "wi1": kmajor(reorder(gi["W_ih1"]), 4).astype(bf),
        "wh1": kmajor(reorder(gi["W_hh1"]), 4).astype(bf),
        "wat": wat.astype(bf),
        "wou": kmajor(gi["W_out"], 4).astype(bf),
        "b0": reorder((gi["b_ih0"] + gi["b_hh0"]).reshape(4 * H, 1))
        .reshape(16, 128)
        .astype(bf),
        "b1": reorder((gi["b_ih1"] + gi["b_hh1"]).reshape(4 * H, 1))
        .reshape(16, 128)
        .astype(bf),
        "batt": gi["b_attn"].reshape(2, 128).astype(bf),
        "wab": wab.reshape(1, ENC).astype(bf),
        "i16": np.repeat(np.eye(16, dtype=np.float32), 8, axis=1).astype(bf),
        "i2": np.repeat(np.eye(2, dtype=np.float32), 8, axis=1).astype(bf),
        "bo": np.repeat(gi["b_out"].reshape(8, 128).T, 8, axis=1).astype(
            np.float32
        ),
    }
    in_maps = []
    encm = gi["encoder2_hiddens"] * (1 - gi["x2_mask"])[:, :, None]
    for c in range(NCORES):
        e = encm[c * BL : (c + 1) * BL]  # [8, 256, 1024]
        enc_t = np.ascontiguousarray(
            e.reshape(BL, 2, 128, IN).transpose(2, 0, 1, 3).reshape(128, BL * 2 * IN)
        ).astype(bf)
        m = gi["x2_mask"][c * BL : (c + 1) * BL]  # [8, 256] int32
        mf = (1 - m).astype(np.float32).T  # [256, 8]
        msk = np.ascontiguousarray(
            mf.reshape(2, 128, BL).transpose(1, 0, 2).reshape(128, 16)
        )
        in_maps.append({**shared, "enc": enc_t, "msk": msk})
    return in_maps


def decode_y(arr, dec=DEC):
    """[8, TCH, 128, TW*8] per-core DRAM layout -> [BL, dec, IN]."""
    tch = max(dec // 32, 1)
    tw = dec // tch
    a = arr.reshape(8, tch, 128, tw, 8)        # (g, tc, p, t', b)
    # y[b, t, g*128+p] with t = tc*tw + t'
    return np.ascontiguousarray(
        a.transpose(4, 1, 3, 0, 2)             # (b, tc, t', g, p)
    ).reshape(BL, dec, IN)


_cache = {}


def _get_nc(dec=DEC, unroll=8, loop_mult=1):
    key = (dec, unroll, loop_mult)
    if key not in _cache:
        _cache[key] = build_nc(dec, unroll, loop_mult)
    return _cache[key]


class Runner:
    """Jit-compiles the Bass program once; repeat calls reuse the executable
    and the device-resident input shards (only fresh output buffers are
    shipped per call when donation is enabled)."""

    def __init__(self, nc, donate=True):
        import jax
        from concourse import bass2jax
        from jax.experimental.shard_map import shard_map
        from jax.sharding import Mesh, PartitionSpec

        bass2jax.install_neuronx_cc_hook()
        self.jax = jax
        self.nc = nc
        self.donate = donate
        pname = nc.partition_id_tensor.name if nc.partition_id_tensor else None
        in_names, out_names, out_avals, zero_outs = [], [], [], []
        self.in_shapes = {}
        for alloc in nc.m.functions[0].allocations:
            if not isinstance(alloc, mybir.MemoryLocationSet):
                continue
            name = alloc.memorylocations[0].name
            if alloc.kind == "ExternalInput":
                if name != pname:
                    in_names.append(name)
                    self.in_shapes[name] = (
                        tuple(alloc.tensor_shape),
                        mybir.dt.np(alloc.dtype),
                    )
            elif alloc.kind == "ExternalOutput":
                shape = tuple(alloc.tensor_shape)
                dtype = mybir.dt.np(alloc.dtype)
                out_names.append(name)
                out_avals.append(jax.core.ShapedArray(shape, dtype))
                zero_outs.append(np.zeros(shape, dtype))
        self.in_names, self.out_names = in_names, out_names
        self.out_avals, self.zero_outs = out_avals, zero_outs
        n_params, n_outs = len(in_names), len(out_names)
        all_names = in_names + out_names + ([pname] if pname else [])

        def _body(*args):
            operands = list(args)
            if pname is not None:
                operands.append(bass2jax.partition_id_tensor())
            outs = bass2jax._bass_exec_p.bind(
                *operands,
                out_avals=tuple(out_avals),
                in_names=tuple(all_names),
                out_names=tuple(out_names),
                lowering_input_output_aliases=(),
                sim_require_finite=True,
                sim_require_nnan=True,
                nc=nc,
            )
            return tuple(outs)

        devices = jax.devices()[:NCORES]
        assert len(devices) == NCORES
        self.mesh = Mesh(np.asarray(devices), ("core",))
        in_specs = (PartitionSpec("core"),) * (n_params + n_outs)
        out_specs = (PartitionSpec("core"),) * n_outs
        kw = (
            dict(donate_argnums=tuple(range(n_params, n_params + n_outs)))
            if donate
            else {}
        )
        self.fn = jax.jit(
            shard_map(
                _body, mesh=self.mesh, in_specs=in_specs, out_specs=out_specs,
                check_rep=False,
            ),
            keep_unused=True,
            **kw,
        )
        self._dev_in = None

    def _globalize(self, in_maps):
        jax = self.jax
        from jax.sharding import NamedSharding, PartitionSpec

        sh = NamedSharding(self.mesh, PartitionSpec("core"))
        arrs = []
        for name in self.in_names:
            if name in in_maps[0]:
                g = np.concatenate(
                    [np.asarray(m[name]) for m in in_maps], axis=0
                )
            else:  # auto-fill (e.g. the cache-bust tensor)
                shape, dt = self.in_shapes[name]
                g = np.zeros((NCORES * shape[0], *shape[1:]), dt)
            arrs.append(jax.device_put(g, sh))
        return arrs

    def set_inputs(self, in_maps):
        self._dev_in = self._globalize(in_maps)

    def _zeros_dev(self):
        from jax.sharding import NamedSharding, PartitionSpec

        sh = NamedSharding(self.mesh, PartitionSpec("core"))
        return [
            self.jax.device_put(
                np.zeros((NCORES * z.shape[0], *z.shape[1:]), z.dtype), sh
            )
            for z in self.zero_outs
        ]

    def __call__(self):
        outs = self.fn(*self._dev_in, *self._zeros_dev())
        return outs

    def gather(self, outs):
        res = []
        for i, name in enumerate(self.out_names):
            a = np.asarray(outs[i])
            res.append(a.reshape(NCORES, *self.out_avals[i].shape))
        return dict(zip(self.out_names, res))


_runner_cache = {}


def get_runner(dec=DEC, unroll=None, donate=True, loop_mult=1):
    unroll = CFG["unroll"] if unroll is None else unroll
    key = (dec, unroll, donate, loop_mult)
    if key not in _runner_cache:
        _runner_cache[key] = Runner(_get_nc(dec, unroll, loop_mult), donate=donate)
    return _runner_cache[key]


CFG = dict(unroll=16)


def run_on_hw(inputs, dec=DEC, unroll=None):
    unroll = CFG["unroll"] if unroll is None else unroll
    r = get_runner(dec, unroll)
    r.set_inputs(prep_inputs(inputs))
    outs = r()
    ys = r.gather(outs)["y"]  # [NCORES, dec+1, 128, 64]
    y = np.concatenate([decode_y(ys[c], dec) for c in range(NCORES)], axis=0)
    return y.astype(np.float32)


def kernel(**inputs):
    return run_on_hw(inputs)
